# revision 3
# baseline (speedup 1.0000x reference)
"""Trainium2 Bass kernel v2 for nn_BasicConv_78915729097031 (e3nn GNN conv).

Math per edge e (i=src, j=dst):
    w_e   = radial_mlp(emb_e)                # [4096] per-edge TP weights
    msg_e = TP(x[i_e], sh_e, w_e)            # [128]
    out[n] = (1/sqrt(8)) * sum_{e: j_e=n} msg_e

Design (DVE-floor): the kernel is bound by the per-edge TP contraction
sum_u av[e,u]*wq[e,u,w] (6144 product terms per 128-edge tile), which runs
as six fused multiply+cumsum custom DVE ops straight from PSUM at the
engine's 1 elem/cycle; every other engine is kept off the critical path:
  * one-hot scatter matrices and all CG/norm folding precomputed on host;
  * MLP1 bf16 + silu -> fp16 h; MLP2 as 8 fp16 512-col matmuls;
  * scans write per-page cumsum boundaries (fp32) directly into the
    scatter rhs; per-w sums are recovered by differencing on the host
    AFTER the linear scatter (cumsum and segment-sum commute);
  * c-path sh1_k scaling runs on ACT via per-partition scale APs;
  * scatter = 2 fp32 matmuls/tile sharing one LDWEIGHTS, accumulating
    [128 nodes x 224 cols] in PSUM per chunk with exactly ONE start=True
    group per chunk (a second start=True in the same bank clears the
    first group's has_written bits and drops its accumulation).
A "pool" plan (ACT fp16 convert + DVE product + segmented tensor_reduce)
is selectable per quarter via KV2_PLAN but measured no faster: this
toolchain's DVE table has no 2x/4x uop rows, so all DVE ops run 1 elem/cyc
regardless of dtype, and GPSIMD (no PSUM access, ~3ns/elem) cannot help.
"""
import os
import sys

import numpy as np

for _p in ("/opt/trn_rl_repo", "/root/.axon_site/_ro/trn_rl_repo"):
    if os.path.isdir(_p) and _p not in sys.path:
        sys.path.insert(0, _p)
        break

MUL = 32
N_NODES = 8192
N_EDGES = 65536
INV_SQRT3 = 1.0 / np.sqrt(3.0)
NORM0 = np.sqrt(1.0 / (2.0 * MUL))
NORM1 = np.sqrt(3.0 / (2.0 * MUL))
SILU_GAIN = 1.6790
NUM_NEIGHBORS = 8.0
NC = 8
NPC = 128
CHUNKS_PER_CORE = (N_NODES // NPC) // NC   # 8

# plan: per-quarter contraction strategy.
#   a/b are always paired: both "pool" (merged 64-wide pages) or both "scan".
#   format: mode:prodengine  with mode in {pool, scan}, prodengine {dve, gps}
#   (pool mode: ACT converts the PSUM quarter to fp16, prodengine does the
#   fp16 multiply, DVE pool_avg reduces; GPSIMD cannot read PSUM.)
_DEFAULT_PLAN = os.environ.get(
    "KV2_PLAN", "ab=scan,c=scan,d=scan")


def _parse_plan(s):
    plan = {}
    for part in s.split(","):
        k, v = part.split("=")
        bits = v.split(":")
        plan[k] = (bits[0], bits[1] if len(bits) > 1 else "act")
    assert set(plan) == {"ab", "c", "d"}
    return plan


# av column layout (fp16): [a(0:32) | b(32:64) | d0 d1 d2 (64:160) |
#                           c(160:192) | sh1 (192:195)]
AV_A, AV_B, AV_D, AV_C, AV_SH1 = 0, 32, 64, 160, 192
AV_COLS = 195

# red (rhs of scatter matmuls) col layout — first 224 cols mirror m_ps so a
# single start=True matmul covers the whole accumulator (two start=True
# groups in one PSUM bank clear each other's has_written):
# [a(0:32) | d0 d1 d2 (32:128) | c'0 c'1 c'2 (128:224) | b (224:256) |
#  c (256:288)]
RED_COLS = 288

# scatter PSUM col layout: [out0 (0:32) | d-part (32:128) | c'-part (128:224)]
M_COLS = 224


# --------------------------------------------------------------------------- #
# Host-side preparation
# --------------------------------------------------------------------------- #
def _host_prep(x, edge_index, edge_attr, edge_len_emb, W1, W2):
    i = edge_index[0].astype(np.int64)
    j = edge_index[1].astype(np.int64)
    E = i.shape[0]
    order = np.argsort(j, kind="stable")
    i_s, j_s = i[order], j[order]
    sh = edge_attr[order].astype(np.float32)
    emb = edge_len_emb[order].astype(np.float32)
    xg = x[i_s].astype(np.float32)
    x0 = xg[:, :MUL]
    x1 = xg[:, MUL:].reshape(E, MUL, 3)
    sh0 = sh[:, 0]
    sh1 = sh[:, 1:4]

    s8 = 1.0 / np.sqrt(NUM_NEIGHBORS)
    av = np.zeros((E, AV_COLS), np.float32)
    av[:, AV_A:AV_A + 32] = x0 * sh0[:, None] * (NORM0 * s8)
    av[:, AV_B:AV_B + 32] = (np.einsum("eui,ei->eu", x1, sh1)
                             * (INV_SQRT3 * NORM0 * s8))
    for k in range(3):
        av[:, AV_D + 32 * k:AV_D + 32 * k + 32] = (
            x1[:, :, k] * (sh0[:, None] * (INV_SQRT3 * NORM1 * s8)))
    av[:, AV_C:AV_C + 32] = x0 * (INV_SQRT3 * NORM1 * s8)
    av[:, AV_SH1:AV_SH1 + 3] = sh1

    W1eff = (W1 / np.sqrt(W1.shape[0])).astype(np.float32)              # [64,128]
    W2eff = (SILU_GAIN * W2 / np.sqrt(W2.shape[0])).astype(np.float32)  # [128,4096]
    # quarter q = path block [a,b,c,d]; within a quarter (w outer, u inner)
    W2eff = (W2eff.reshape(128, 4, MUL, MUL)      # [h, path, u, w]
             .transpose(0, 1, 3, 2)               # [h, path, w, u]
             .reshape(128, 4096).copy())

    n_chunks = N_NODES // NPC
    chunk_of_edge = j_s // NPC
    counts = np.bincount(chunk_of_edge, minlength=n_chunks)
    tiles_of_chunk = np.maximum(1, np.ceil(counts / 128).astype(np.int64))

    order2 = np.argsort(-tiles_of_chunk, kind="stable")
    assign = np.empty((NC, CHUNKS_PER_CORE), np.int64)
    for s in range(CHUNKS_PER_CORE):
        row = order2[s * NC:(s + 1) * NC]
        assign[:, s] = row if s % 2 == 0 else row[::-1]
    schedule = tuple(int(tiles_of_chunk[assign[:, s]].max())
                     for s in range(CHUNKS_PER_CORE))
    slot_base = np.concatenate([[0], np.cumsum(np.array(schedule) * 128)])
    e_pad = int(slot_base[-1])
    n_tiles = sum(schedule)

    import ml_dtypes
    bf16 = ml_dtypes.bfloat16
    embT = np.zeros((NC, n_tiles, 64, 128), bf16)
    avx = np.zeros((NC, e_pad, AV_COLS), np.float16)
    sh1f = np.zeros((NC, e_pad, 3), np.float32)
    s16 = np.zeros((NC, n_tiles, 128, 128), np.float32)
    starts = np.concatenate([[0], np.cumsum(counts)])
    eye = np.eye(128, dtype=np.float32)
    for core in range(NC):
        for s in range(CHUNKS_PER_CORE):
            c = int(assign[core, s])
            lo, hi = int(starts[c]), int(starts[c + 1])
            cnt = hi - lo
            base = int(slot_base[s])
            et = emb[lo:hi].T                      # [64, cnt]
            etp = np.zeros((64, schedule[s] * 128), np.float32)
            etp[:, :cnt] = et
            t0 = sum(schedule[:s])
            embT[core, t0:t0 + schedule[s]] = (
                etp.reshape(64, schedule[s], 128).transpose(1, 0, 2)
                .astype(bf16))
            avx[core, base:base + cnt] = av[lo:hi].astype(np.float16)
            sh1f[core, base:base + cnt] = sh1[lo:hi]
            nloc = (j_s[lo:hi] - c * NPC).astype(np.int64)
            onehot = np.zeros((schedule[s] * 128, 128), np.float32)
            onehot[np.arange(cnt)] = eye[nloc]
            s16[core, t0:t0 + schedule[s]] = (
                onehot.reshape(schedule[s], 128, 128))
    return dict(embT=embT, avx=avx, sh1f=sh1f, s16=s16,
                W1eff=W1eff.astype(bf16), W2eff=W2eff.astype(np.float16),
                schedule=schedule, e_pad=e_pad, n_tiles=n_tiles,
                assign=assign)


# --------------------------------------------------------------------------- #
# Custom DVE op (fused multiply + running cumsum; boundary-only writes)
# --------------------------------------------------------------------------- #
_SCAN_OP_NAME = "TT_MUL_CUMSUM_ANT"


def _register_scan_op():
    import concourse.dve_ops as dve_ops
    for o in dve_ops.OPS:
        if o.name == _SCAN_OP_NAME:
            return o
    from concourse.dve_spec import Spec, Src0, Src1, scan, AluOp, lower, _has_src1
    from concourse.dve_uop import DveOpSpec

    def _ref(in0, in1, s0, s1, imm2):
        prod = in0.astype(np.float32) * in1.astype(np.float32)
        flat = prod.reshape(prod.shape[0], -1)
        return np.cumsum(flat, axis=-1).reshape(prod.shape)

    spec = Spec(body=scan(AluOp.ADD, Src0 * Src1), reference=_ref)
    shas = {}
    for ver in ("v3", "v4"):
        tmp = DveOpSpec(name=_SCAN_OP_NAME, opcode=0, uops=lower(spec, ver=ver),
                        rd1_en=_has_src1(spec))
        shas[ver] = tmp.sha(ver)
    op = dve_ops.DveOp(_SCAN_OP_NAME, spec, subdim=True, uops_sha=shas)
    dve_ops.OPS.append(op)
    dve_ops._SUB_OPCODE_FOR_NAME[_SCAN_OP_NAME] = (
        dve_ops._CUSTOM_DVE_ROW_BASE + len(dve_ops.OPS) - 1)
    dve_ops.CUSTOM_DVE_SPECS[_SCAN_OP_NAME] = spec
    return op


# --------------------------------------------------------------------------- #
# Bass program
# --------------------------------------------------------------------------- #
_PROGRAM_CACHE = {}


def _build_program(schedule, e_pad, plan_s, repeat=1):
    key = (schedule, e_pad, plan_s, repeat)
    if key in _PROGRAM_CACHE:
        return _PROGRAM_CACHE[key]
    plan = _parse_plan(plan_s)

    from concourse import bacc, mybir
    import concourse.tile as tile

    scan_op = _register_scan_op()

    f32 = mybir.dt.float32
    f16 = mybir.dt.float16
    bf16 = mybir.dt.bfloat16
    AF = mybir.ActivationFunctionType
    OP = mybir.AluOpType

    nc = bacc.Bacc("TRN2", target_bir_lowering=False, debug=False,
                   num_devices=NC)

    n_tiles = sum(schedule)
    embT_d = nc.dram_tensor("embT", [n_tiles, 64, 128], bf16,
                            kind="ExternalInput").ap()
    avx_d = nc.dram_tensor("avx", [e_pad, AV_COLS], f16,
                           kind="ExternalInput").ap()
    s_d = nc.dram_tensor("s16", [n_tiles, 128, 128], f32,
                         kind="ExternalInput").ap()
    sh1_d = nc.dram_tensor("sh1f", [e_pad, 3], f32,
                           kind="ExternalInput").ap()
    w1_d = nc.dram_tensor("w1", [64, 128], bf16, kind="ExternalInput").ap()
    w2_d = nc.dram_tensor("w2", [128, 4096], f16, kind="ExternalInput").ap()
    out_d = nc.dram_tensor("out", [CHUNKS_PER_CORE * 128, M_COLS], f32,
                           kind="ExternalOutput").ap()

    with tile.TileContext(nc) as tc:
        with (
            tc.tile_pool(name="const", bufs=1) as const_p,
            tc.tile_pool(name="inp", bufs=4) as inp_p,
            tc.tile_pool(name="hsb", bufs=3) as h_p,
            tc.tile_pool(name="cnv", bufs=2) as cnv_p,
            tc.tile_pool(name="prd", bufs=2) as prd_p,
            tc.tile_pool(name="red", bufs=3) as red_p,
            tc.tile_pool(name="osb", bufs=2) as out_p,
            tc.tile_pool(name="hps", bufs=1, space="PSUM") as hps_p,
            tc.tile_pool(name="wps", bufs=3, space="PSUM") as wps_p,
            tc.tile_pool(name="mps", bufs=1, space="PSUM") as mps_p,
        ):
            w1_sb = const_p.tile([64, 128], bf16)
            nc.sync.dma_start(w1_sb[:], w1_d[:])
            w2_sb = const_p.tile([128, 4096], f16)
            nc.sync.dma_start(w2_sb[:], w2_d[:])

            for cc_rep in range(CHUNKS_PER_CORE * repeat):
                cc = cc_rep % CHUNKS_PER_CORE
                m_ps = mps_p.tile([128, M_COLS], f32, space="PSUM", tag="m")
                tpc = schedule[cc]
                t_base = sum(schedule[:cc])
                for t in range(tpc):
                    til = t_base + t
                    e0 = til * 128
                    first, last = t == 0, t == tpc - 1

                    # ---- loads ----
                    emb_sb = inp_p.tile([64, 128], bf16, tag="emb")
                    nc.sync.dma_start(emb_sb[:], embT_d[til])
                    av_sb = inp_p.tile([128, AV_COLS], f16, tag="av")
                    nc.sync.dma_start(av_sb[:], avx_d[e0:e0 + 128, :])
                    s_sb = inp_p.tile([128, 128], f32, tag="s")
                    nc.sync.dma_start(s_sb[:], s_d[til])
                    sh1_sb = inp_p.tile([128, 3], f32, tag="sh1")
                    nc.sync.dma_start(sh1_sb[:], sh1_d[e0:e0 + 128, :])

                    # ---- MLP1 + silu -> h [128h, 128e] fp16 ----
                    hpre = hps_p.tile([128, 128], f32, space="PSUM",
                                      tag="hpre")
                    nc.tensor.matmul(hpre[:], lhsT=w1_sb[:], rhs=emb_sb[:],
                                     start=True, stop=True)
                    h_sb = h_p.tile([128, 128], f16, tag="h")
                    nc.scalar.activation(h_sb[:], hpre[:], AF.Silu)

                    red = red_p.tile([128, RED_COLS], f32, tag="red")

                    # red block targets per quarter:
                    #   merged-ab pool -> red[:,0:32]; scan-a -> 0:32,
                    #   scan-b -> 128:160; d -> 32:128; c -> 160:192
                    def scan_quarter(wq_ps, av_off, n_pages, red_off, in0_ap):
                        nc.vector._custom_dve(
                            scan_op,
                            out=red[:, red_off:red_off + n_pages]
                                .rearrange("p w -> p w ()")
                                .to_broadcast([128, n_pages, 32]),
                            in0=in0_ap,
                            in1=av_sb[:, av_off:av_off + 32]
                                .rearrange("p u -> p () u")
                                .to_broadcast([128, n_pages, 32]),
                        )

                    conv_tiles = {}
                    for q, qn in enumerate(("a", "b", "c", "d")):
                        wq_ps = wps_p.tile([128, 1024], f32, space="PSUM",
                                           tag="wq")
                        for half in range(2):
                            sl = slice(half * 512, half * 512 + 512)
                            nc.tensor.matmul(
                                wq_ps[:, sl], lhsT=h_sb[:],
                                rhs=w2_sb[:, q * 1024 + half * 512:
                                          q * 1024 + half * 512 + 512],
                                start=True, stop=True)
                        pk = "ab" if qn in ("a", "b") else qn
                        mode, peng = plan[pk]
                        if mode == "pool":
                            if pk == "ab":
                                if "ab" not in conv_tiles:
                                    conv_tiles["ab"] = cnv_p.tile(
                                        [128, 2048], f16, tag="cab",
                                        name="cv_ab")
                                cv = conv_tiles["ab"]
                                dst = cv[:, 0:1024] if qn == "a" \
                                    else cv[:, 1024:2048]
                            else:
                                cv = cnv_p.tile([128, 1024], f16,
                                                tag="c" + qn,
                                                name="cv_" + qn)
                                conv_tiles[pk] = cv
                                dst = cv[:]
                            nc.scalar.copy(out=dst, in_=wq_ps[:])
                        else:
                            # scan straight from PSUM
                            in0 = wq_ps[:].rearrange("p (w u) -> p w u", w=32)
                            if qn == "a":
                                scan_quarter(wq_ps, AV_A, 32, 0, in0)
                            elif qn == "b":
                                scan_quarter(wq_ps, AV_B, 32, 224, in0)
                            elif qn == "c":
                                scan_quarter(wq_ps, AV_C, 32, 256, in0)
                            else:
                                for k in range(3):
                                    scan_quarter(wq_ps, AV_D + 32 * k, 32,
                                                 32 + 32 * k, in0)

                    # ---- fp16 product + segmented-reduce paths ----
                    def seg_reduce(dst, src_ap):
                        with nc.allow_low_precision("fp16 page sums"):
                            nc.vector.tensor_reduce(
                                out=dst, in_=src_ap,
                                axis=mybir.AxisListType.X, op=OP.add,
                                opt_input=False, opt_output=False)

                    def prod_engine(name):
                        return nc.vector if name == "dve" else nc.gpsimd

                    if plan["ab"][0] == "pool":
                        cv = conv_tiles["ab"]
                        pr_ab = prd_p.tile([128, 2048], f16, tag="pab",
                                           name="pr_ab")
                        prod_engine(plan["ab"][1]).tensor_tensor(
                            out=pr_ab[:].rearrange("p (w h u) -> p w h u",
                                                   h=2, u=32),
                            in0=cv[:].rearrange("p (h w u) -> p w h u",
                                                h=2, u=32),
                            in1=av_sb[:, AV_A:AV_A + 64]
                                .rearrange("p (h u) -> p () h u", h=2)
                                .to_broadcast([128, 32, 2, 32]),
                            op=OP.mult)
                        seg_reduce(red[:, 0:32],
                                   pr_ab[:].rearrange("p (w v) -> p w v",
                                                      v=64))
                    if plan["c"][0] == "pool":
                        cv = conv_tiles["c"]
                        pr_c = prd_p.tile([128, 1024], f16, tag="pc",
                                          name="pr_c")
                        prod_engine(plan["c"][1]).tensor_tensor(
                            out=pr_c[:].rearrange("p (w u) -> p w u", w=32),
                            in0=cv[:].rearrange("p (w u) -> p w u", w=32),
                            in1=av_sb[:, AV_C:AV_C + 32]
                                .rearrange("p u -> p () u")
                                .to_broadcast([128, 32, 32]),
                            op=OP.mult)
                        seg_reduce(red[:, 256:288],
                                   pr_c[:].rearrange("p (w u) -> p w u",
                                                     w=32))
                    if plan["d"][0] == "pool":
                        cv = conv_tiles["d"]
                        pr_d = prd_p.tile([128, 3072], f16, tag="pd",
                                          name="pr_d")
                        prod_engine(plan["d"][1]).tensor_tensor(
                            out=pr_d[:].rearrange("p (k w u) -> p k w u",
                                                  k=3, u=32),
                            in0=cv[:].rearrange("p (w u) -> p () w u", w=32)
                                .to_broadcast([128, 3, 32, 32]),
                            in1=av_sb[:, AV_D:AV_D + 96]
                                .rearrange("p (k u) -> p k () u", k=3)
                                .to_broadcast([128, 3, 32, 32]),
                            op=OP.mult)
                        seg_reduce(red[:, 32:128],
                                   pr_d[:].rearrange("p (kw u) -> p kw u",
                                                     u=32))

                    # ---- c' = c-block * sh1_k (ACT, per-partition scale) --
                    for k in range(3):
                        nc.scalar.activation(
                            red[:, 128 + 32 * k:160 + 32 * k],
                            red[:, 256:288],
                            AF.Copy,
                            scale=sh1_sb[:, k:k + 1])

                    # ---- scatter (one lhsT; PSUM-accumulated over chunk;
                    #      exactly ONE start=True matmul per chunk) ----
                    nc.tensor.matmul(m_ps[:, 0:224], lhsT=s_sb[:],
                                     rhs=red[:, 0:224],
                                     start=first,
                                     stop=(last and plan["ab"][0] != "scan"),
                                     skip_group_check=True)
                    if plan["ab"][0] == "scan":
                        nc.tensor.matmul(m_ps[:, 0:32], lhsT=s_sb[:],
                                         rhs=red[:, 224:256],
                                         start=False, stop=last,
                                         skip_group_check=True)

                # ---- store chunk ----
                o_sb = out_p.tile([128, M_COLS], f32, tag="o")
                nc.scalar.copy(out=o_sb[:], in_=m_ps[:])
                nc.sync.dma_start(out_d[cc * 128:(cc + 1) * 128, :], o_sb[:])

    nc.compile()
    _PROGRAM_CACHE[key] = nc
    return nc


# --------------------------------------------------------------------------- #
# Entry point
# --------------------------------------------------------------------------- #
def _build_in_maps(prep):
    in_maps = []
    for c in range(NC):
        in_maps.append({
            "embT": prep["embT"][c],
            "avx": prep["avx"][c],
            "s16": prep["s16"][c],
            "sh1f": prep["sh1f"][c],
            "w1": prep["W1eff"],
            "w2": prep["W2eff"],
        })
    return in_maps


def _postprocess(per_core_out, assign, plan_s=None):
    plan = _parse_plan(plan_s if plan_s is not None else _DEFAULT_PLAN)
    M = np.empty((N_NODES, M_COLS), np.float32)
    for core in range(NC):
        for s in range(CHUNKS_PER_CORE):
            c = int(assign[core, s])
            M[c * NPC:(c + 1) * NPC] = per_core_out[core][s * NPC:(s + 1) * NPC]

    def blkdiff(B):
        # per-32-block de-cumsum along columns
        out = B.copy()
        out[:, 1:] -= B[:, :-1]
        return out

    out0 = M[:, 0:32].copy()
    dpart = M[:, 32:128].reshape(N_NODES, 3, 32)
    cpart = M[:, 128:224].reshape(N_NODES, 3, 32)
    if plan["ab"][0] == "scan":
        out0 = blkdiff(out0)          # cum(a)+cum(b) both reset at col 0
    if plan["d"][0] == "scan":
        dpart = np.concatenate(
            [blkdiff(dpart[:, k])[:, None, :] for k in range(3)], axis=1)
    if plan["c"][0] == "scan":
        cpart = np.concatenate(
            [blkdiff(cpart[:, k])[:, None, :] for k in range(3)], axis=1)
    out1 = dpart + cpart              # [N, 3, 32] (k, w)
    out = np.empty((N_NODES, 128), np.float32)
    out[:, :32] = out0
    out[:, 32:] = out1.transpose(0, 2, 1).reshape(N_NODES, 96)  # (w, k)
    return out


def _prepare(x, edge_index, edge_attr, edge_len_emb, W1, W2, repeat=1,
             plan_s=None):
    if plan_s is None:
        plan_s = _DEFAULT_PLAN
    x = np.asarray(x, np.float32)
    edge_index = np.asarray(edge_index)
    edge_attr = np.asarray(edge_attr, np.float32)
    edge_len_emb = np.asarray(edge_len_emb, np.float32)
    W1 = np.asarray(W1, np.float32)
    W2 = np.asarray(W2, np.float32)
    prep = _host_prep(x, edge_index, edge_attr, edge_len_emb, W1, W2)
    nc = _build_program(prep["schedule"], prep["e_pad"], plan_s, repeat=repeat)
    return prep, nc, _build_in_maps(prep)


def kernel(x, edge_index, edge_attr, edge_len_emb, W1, W2, _results_out=None):
    prep, nc, in_maps = _prepare(x, edge_index, edge_attr, edge_len_emb,
                                 W1, W2)

    from concourse.bass_utils import run_bass_kernel_spmd

    res = run_bass_kernel_spmd(nc, in_maps, core_ids=list(range(NC)))
    if _results_out is not None:
        _results_out.append(res)

    return _postprocess([res.results[c]["out"] for c in range(NC)],
                        prep["assign"], _DEFAULT_PLAN)


# revision 4
# speedup vs baseline: 1.1522x; 1.1522x over previous
"""Trainium2 Bass kernel v2 for nn_BasicConv_78915729097031 (e3nn GNN conv).

Math per edge e (i=src, j=dst):
    w_e   = radial_mlp(emb_e)                # [4096] per-edge TP weights
    msg_e = TP(x[i_e], sh_e, w_e)            # [128]
    out[n] = (1/sqrt(8)) * sum_{e: j_e=n} msg_e

Design (DVE-floor): the kernel is bound by the per-edge TP contraction
sum_u av[e,u]*wq[e,u,w] (6144 product terms per 128-edge tile), which runs
as six fused multiply+cumsum custom DVE ops straight from PSUM at the
engine's 1 elem/cycle; every other engine is kept off the critical path:
  * one-hot scatter matrices and all CG/norm folding precomputed on host;
  * MLP1 bf16 + silu -> fp16 h; MLP2 as 8 fp16 512-col matmuls;
  * scans write per-page cumsum boundaries (fp32) directly into the
    scatter rhs; per-w sums are recovered by differencing on the host
    AFTER the linear scatter (cumsum and segment-sum commute);
  * c-path sh1_k scaling runs on ACT via per-partition scale APs;
  * scatter = 2 fp32 matmuls/tile sharing one LDWEIGHTS, accumulating
    [128 nodes x 224 cols] in PSUM per chunk with exactly ONE start=True
    group per chunk (a second start=True in the same bank clears the
    first group's has_written bits and drops its accumulation).
A "pool" plan (ACT fp16 convert + DVE product + segmented tensor_reduce)
is selectable per quarter via KV2_PLAN but measured no faster: this
toolchain's DVE table has no 2x/4x uop rows, so all DVE ops run 1 elem/cyc
regardless of dtype, and GPSIMD (no PSUM access, ~3ns/elem) cannot help.
"""
import os
import sys

import numpy as np

for _p in ("/opt/trn_rl_repo", "/root/.axon_site/_ro/trn_rl_repo"):
    if os.path.isdir(_p) and _p not in sys.path:
        sys.path.insert(0, _p)
        break

MUL = 32
N_NODES = 8192
N_EDGES = 65536
INV_SQRT3 = 1.0 / np.sqrt(3.0)
NORM0 = np.sqrt(1.0 / (2.0 * MUL))
NORM1 = np.sqrt(3.0 / (2.0 * MUL))
SILU_GAIN = 1.6790
NUM_NEIGHBORS = 8.0
NC = 8
NPC = 128
CHUNKS_PER_CORE = (N_NODES // NPC) // NC   # 8

# plan: per-quarter contraction strategy.
#   a/b are always paired: both "pool" (merged 64-wide pages) or both "scan".
#   format: mode:prodengine  with mode in {pool, scan}, prodengine {dve, gps}
#   (pool mode: ACT converts the PSUM quarter to fp16, prodengine does the
#   fp16 multiply, DVE pool_avg reduces; GPSIMD cannot read PSUM.)
_DEFAULT_PLAN = os.environ.get(
    "KV2_PLAN", "ab=scan,c=scan,d=scan")


def _parse_plan(s):
    plan = {}
    for part in s.split(","):
        k, v = part.split("=")
        bits = v.split(":")
        plan[k] = (bits[0], bits[1] if len(bits) > 1 else "act")
    assert set(plan) == {"ab", "c", "d"}
    return plan


# av column layout (fp16): [a(0:32) | b(32:64) | d0 d1 d2 (64:160) |
#                           c(160:192) | sh1 (192:195)]
AV_A, AV_B, AV_D, AV_C, AV_SH1 = 0, 32, 64, 160, 192
AV_COLS = 195

# red (rhs of scatter matmuls) col layout — first 224 cols mirror m_ps so a
# single start=True matmul covers the whole accumulator (two start=True
# groups in one PSUM bank clear each other's has_written):
# [a(0:32) | d0 d1 d2 (32:128) | c'0 c'1 c'2 (128:224) | b (224:256) |
#  c (256:288)]
RED_COLS = 288

# scatter PSUM col layout: [out0 (0:32) | d-part (32:128) | c'-part (128:224)]
M_COLS = 224


# --------------------------------------------------------------------------- #
# Host-side preparation
# --------------------------------------------------------------------------- #
def _balance_nodes(j):
    """Relabel nodes so each 128-node chunk has <= 1024 incoming edges.

    Greedy LPT bin-packing by in-degree + a swap repair pass. Returns
    node_perm with node_perm[new_id] = original node id.
    """
    deg = np.bincount(j, minlength=N_NODES).astype(np.int64)
    nbins = N_NODES // NPC
    cap = NPC * 8
    bin_e = np.zeros(nbins, np.int64)
    bin_n = np.zeros(nbins, np.int64)
    members = [[] for _ in range(nbins)]
    big = np.int64(1 << 60)
    for nd in np.argsort(-deg, kind="stable"):
        be = np.where(bin_n < NPC, bin_e, big)
        b = int(be.argmin())
        members[b].append(int(nd))
        bin_e[b] += deg[nd]
        bin_n[b] += 1
    for _ in range(300):
        over = np.where(bin_e > cap)[0]
        if not len(over):
            break
        progressed = False
        for ob in over:
            x = int(bin_e[ob] - cap)
            if x <= 0:
                continue
            degs_ob = {}
            for a in members[ob]:
                degs_ob.setdefault(int(deg[a]), a)
            done = False
            for ub in np.argsort(bin_e):
                y = int(cap - bin_e[ub])
                if y <= 0:
                    continue
                hi = min(x, y)
                degs_ub = {}
                for a in members[ub]:
                    degs_ub.setdefault(int(deg[a]), a)
                for delta in range(hi, 0, -1):
                    for db, bnode in degs_ub.items():
                        da = db + delta
                        if da in degs_ob:
                            anode = degs_ob[da]
                            members[ob].remove(anode)
                            members[ub].remove(bnode)
                            members[ob].append(bnode)
                            members[ub].append(anode)
                            bin_e[ob] -= delta
                            bin_e[ub] += delta
                            done = True
                            progressed = True
                            break
                    if done:
                        break
                if done:
                    break
        if not progressed:
            break
    node_perm = np.concatenate([np.array(m, np.int64) for m in members])
    return node_perm


def _host_prep(x, edge_index, edge_attr, edge_len_emb, W1, W2):
    i = edge_index[0].astype(np.int64)
    j_orig = edge_index[1].astype(np.int64)
    E = i.shape[0]
    node_perm = _balance_nodes(j_orig)
    newid = np.empty(N_NODES, np.int64)
    newid[node_perm] = np.arange(N_NODES)
    j = newid[j_orig]
    order = np.argsort(j, kind="stable")
    i_s, j_s = i[order], j[order]
    sh = edge_attr[order].astype(np.float32)
    emb = edge_len_emb[order].astype(np.float32)
    xg = x[i_s].astype(np.float32)
    x0 = xg[:, :MUL]
    x1 = xg[:, MUL:].reshape(E, MUL, 3)
    sh0 = sh[:, 0]
    sh1 = sh[:, 1:4]

    s8 = 1.0 / np.sqrt(NUM_NEIGHBORS)
    av = np.zeros((E, AV_COLS), np.float32)
    av[:, AV_A:AV_A + 32] = x0 * sh0[:, None] * (NORM0 * s8)
    av[:, AV_B:AV_B + 32] = (np.einsum("eui,ei->eu", x1, sh1)
                             * (INV_SQRT3 * NORM0 * s8))
    for k in range(3):
        av[:, AV_D + 32 * k:AV_D + 32 * k + 32] = (
            x1[:, :, k] * (sh0[:, None] * (INV_SQRT3 * NORM1 * s8)))
    av[:, AV_C:AV_C + 32] = x0 * (INV_SQRT3 * NORM1 * s8)
    av[:, AV_SH1:AV_SH1 + 3] = sh1

    W1eff = (W1 / np.sqrt(W1.shape[0])).astype(np.float32)              # [64,128]
    W2eff = (SILU_GAIN * W2 / np.sqrt(W2.shape[0])).astype(np.float32)  # [128,4096]
    # quarter q = path block [a,b,c,d]; within a quarter (w outer, u inner)
    W2eff = (W2eff.reshape(128, 4, MUL, MUL)      # [h, path, u, w]
             .transpose(0, 1, 3, 2)               # [h, path, w, u]
             .reshape(128, 4096).copy())

    n_chunks = N_NODES // NPC
    chunk_of_edge = j_s // NPC
    counts = np.bincount(chunk_of_edge, minlength=n_chunks)
    tiles_of_chunk = np.maximum(1, np.ceil(counts / 128).astype(np.int64))

    order2 = np.argsort(-tiles_of_chunk, kind="stable")
    assign = np.empty((NC, CHUNKS_PER_CORE), np.int64)
    for s in range(CHUNKS_PER_CORE):
        row = order2[s * NC:(s + 1) * NC]
        assign[:, s] = row if s % 2 == 0 else row[::-1]
    schedule = tuple(int(tiles_of_chunk[assign[:, s]].max())
                     for s in range(CHUNKS_PER_CORE))
    slot_base = np.concatenate([[0], np.cumsum(np.array(schedule) * 128)])
    e_pad = int(slot_base[-1])
    n_tiles = sum(schedule)

    import ml_dtypes
    bf16 = ml_dtypes.bfloat16
    embT = np.zeros((NC, n_tiles, 64, 128), bf16)
    avx = np.zeros((NC, e_pad, AV_COLS), np.float16)
    sh1f = np.zeros((NC, e_pad, 3), np.float32)
    s16 = np.zeros((NC, n_tiles, 128, 128), np.float32)
    starts = np.concatenate([[0], np.cumsum(counts)])
    eye = np.eye(128, dtype=np.float32)
    for core in range(NC):
        for s in range(CHUNKS_PER_CORE):
            c = int(assign[core, s])
            lo, hi = int(starts[c]), int(starts[c + 1])
            cnt = hi - lo
            base = int(slot_base[s])
            et = emb[lo:hi].T                      # [64, cnt]
            etp = np.zeros((64, schedule[s] * 128), np.float32)
            etp[:, :cnt] = et
            t0 = sum(schedule[:s])
            embT[core, t0:t0 + schedule[s]] = (
                etp.reshape(64, schedule[s], 128).transpose(1, 0, 2)
                .astype(bf16))
            avx[core, base:base + cnt] = av[lo:hi].astype(np.float16)
            sh1f[core, base:base + cnt] = sh1[lo:hi]
            nloc = (j_s[lo:hi] - c * NPC).astype(np.int64)
            onehot = np.zeros((schedule[s] * 128, 128), np.float32)
            onehot[np.arange(cnt)] = eye[nloc]
            s16[core, t0:t0 + schedule[s]] = (
                onehot.reshape(schedule[s], 128, 128))
    return dict(embT=embT, avx=avx, sh1f=sh1f, s16=s16,
                W1eff=W1eff.astype(bf16), W2eff=W2eff.astype(np.float16),
                schedule=schedule, e_pad=e_pad, n_tiles=n_tiles,
                assign=(assign, node_perm))


# --------------------------------------------------------------------------- #
# Custom DVE op (fused multiply + running cumsum; boundary-only writes)
# --------------------------------------------------------------------------- #
_SCAN_OP_NAME = "TT_MUL_CUMSUM_ANT"


def _register_scan_op():
    import concourse.dve_ops as dve_ops
    for o in dve_ops.OPS:
        if o.name == _SCAN_OP_NAME:
            return o
    from concourse.dve_spec import Spec, Src0, Src1, scan, AluOp, lower, _has_src1
    from concourse.dve_uop import DveOpSpec

    def _ref(in0, in1, s0, s1, imm2):
        prod = in0.astype(np.float32) * in1.astype(np.float32)
        flat = prod.reshape(prod.shape[0], -1)
        return np.cumsum(flat, axis=-1).reshape(prod.shape)

    spec = Spec(body=scan(AluOp.ADD, Src0 * Src1), reference=_ref)
    shas = {}
    for ver in ("v3", "v4"):
        tmp = DveOpSpec(name=_SCAN_OP_NAME, opcode=0, uops=lower(spec, ver=ver),
                        rd1_en=_has_src1(spec))
        shas[ver] = tmp.sha(ver)
    op = dve_ops.DveOp(_SCAN_OP_NAME, spec, subdim=True, uops_sha=shas)
    dve_ops.OPS.append(op)
    dve_ops._SUB_OPCODE_FOR_NAME[_SCAN_OP_NAME] = (
        dve_ops._CUSTOM_DVE_ROW_BASE + len(dve_ops.OPS) - 1)
    dve_ops.CUSTOM_DVE_SPECS[_SCAN_OP_NAME] = spec
    return op


# --------------------------------------------------------------------------- #
# Bass program
# --------------------------------------------------------------------------- #
_PROGRAM_CACHE = {}


def _build_program(schedule, e_pad, plan_s, repeat=1):
    key = (schedule, e_pad, plan_s, repeat)
    if key in _PROGRAM_CACHE:
        return _PROGRAM_CACHE[key]
    plan = _parse_plan(plan_s)

    from concourse import bacc, mybir
    import concourse.tile as tile

    scan_op = _register_scan_op()

    f32 = mybir.dt.float32
    f16 = mybir.dt.float16
    bf16 = mybir.dt.bfloat16
    AF = mybir.ActivationFunctionType
    OP = mybir.AluOpType

    nc = bacc.Bacc("TRN2", target_bir_lowering=False, debug=False,
                   num_devices=NC)

    n_tiles = sum(schedule)
    embT_d = nc.dram_tensor("embT", [n_tiles, 64, 128], bf16,
                            kind="ExternalInput").ap()
    avx_d = nc.dram_tensor("avx", [e_pad, AV_COLS], f16,
                           kind="ExternalInput").ap()
    s_d = nc.dram_tensor("s16", [n_tiles, 128, 128], f32,
                         kind="ExternalInput").ap()
    sh1_d = nc.dram_tensor("sh1f", [e_pad, 3], f32,
                           kind="ExternalInput").ap()
    w1_d = nc.dram_tensor("w1", [64, 128], bf16, kind="ExternalInput").ap()
    w2_d = nc.dram_tensor("w2", [128, 4096], f16, kind="ExternalInput").ap()
    out_d = nc.dram_tensor("out", [CHUNKS_PER_CORE * 128, M_COLS], f32,
                           kind="ExternalOutput").ap()

    with tile.TileContext(nc) as tc:
        with (
            tc.tile_pool(name="const", bufs=1) as const_p,
            tc.tile_pool(name="inp", bufs=4) as inp_p,
            tc.tile_pool(name="hsb", bufs=3) as h_p,
            tc.tile_pool(name="cnv", bufs=2) as cnv_p,
            tc.tile_pool(name="prd", bufs=2) as prd_p,
            tc.tile_pool(name="red", bufs=3) as red_p,
            tc.tile_pool(name="osb", bufs=2) as out_p,
            tc.tile_pool(name="hps", bufs=1, space="PSUM") as hps_p,
            tc.tile_pool(name="wps", bufs=3, space="PSUM") as wps_p,
            tc.tile_pool(name="mps", bufs=1, space="PSUM") as mps_p,
        ):
            w1_sb = const_p.tile([64, 128], bf16)
            nc.sync.dma_start(w1_sb[:], w1_d[:])
            w2_sb = const_p.tile([128, 4096], f16)
            nc.sync.dma_start(w2_sb[:], w2_d[:])

            for cc_rep in range(CHUNKS_PER_CORE * repeat):
                cc = cc_rep % CHUNKS_PER_CORE
                m_ps = mps_p.tile([128, M_COLS], f32, space="PSUM", tag="m")
                tpc = schedule[cc]
                t_base = sum(schedule[:cc])
                for t in range(tpc):
                    til = t_base + t
                    e0 = til * 128
                    first, last = t == 0, t == tpc - 1

                    # ---- loads ----
                    emb_sb = inp_p.tile([64, 128], bf16, tag="emb")
                    nc.sync.dma_start(emb_sb[:], embT_d[til])
                    av_sb = inp_p.tile([128, AV_COLS], f16, tag="av")
                    nc.sync.dma_start(av_sb[:], avx_d[e0:e0 + 128, :])
                    s_sb = inp_p.tile([128, 128], f32, tag="s")
                    nc.sync.dma_start(s_sb[:], s_d[til])
                    sh1_sb = inp_p.tile([128, 3], f32, tag="sh1")
                    nc.sync.dma_start(sh1_sb[:], sh1_d[e0:e0 + 128, :])

                    # ---- MLP1 + silu -> h [128h, 128e] fp16 ----
                    hpre = hps_p.tile([128, 128], f32, space="PSUM",
                                      tag="hpre")
                    nc.tensor.matmul(hpre[:], lhsT=w1_sb[:], rhs=emb_sb[:],
                                     start=True, stop=True)
                    h_sb = h_p.tile([128, 128], f16, tag="h")
                    nc.scalar.activation(h_sb[:], hpre[:], AF.Silu)

                    red = red_p.tile([128, RED_COLS], f32, tag="red")

                    # red block targets per quarter:
                    #   merged-ab pool -> red[:,0:32]; scan-a -> 0:32,
                    #   scan-b -> 128:160; d -> 32:128; c -> 160:192
                    def scan_quarter(wq_ps, av_off, n_pages, red_off, in0_ap):
                        nc.vector._custom_dve(
                            scan_op,
                            out=red[:, red_off:red_off + n_pages]
                                .rearrange("p w -> p w ()")
                                .to_broadcast([128, n_pages, 32]),
                            in0=in0_ap,
                            in1=av_sb[:, av_off:av_off + 32]
                                .rearrange("p u -> p () u")
                                .to_broadcast([128, n_pages, 32]),
                        )

                    conv_tiles = {}
                    for q, qn in enumerate(("a", "b", "c", "d")):
                        wq_ps = wps_p.tile([128, 1024], f32, space="PSUM",
                                           tag="wq")
                        for half in range(2):
                            sl = slice(half * 512, half * 512 + 512)
                            nc.tensor.matmul(
                                wq_ps[:, sl], lhsT=h_sb[:],
                                rhs=w2_sb[:, q * 1024 + half * 512:
                                          q * 1024 + half * 512 + 512],
                                start=True, stop=True)
                        pk = "ab" if qn in ("a", "b") else qn
                        mode, peng = plan[pk]
                        if mode == "pool":
                            if pk == "ab":
                                if "ab" not in conv_tiles:
                                    conv_tiles["ab"] = cnv_p.tile(
                                        [128, 2048], f16, tag="cab",
                                        name="cv_ab")
                                cv = conv_tiles["ab"]
                                dst = cv[:, 0:1024] if qn == "a" \
                                    else cv[:, 1024:2048]
                            else:
                                cv = cnv_p.tile([128, 1024], f16,
                                                tag="c" + qn,
                                                name="cv_" + qn)
                                conv_tiles[pk] = cv
                                dst = cv[:]
                            nc.scalar.copy(out=dst, in_=wq_ps[:])
                        else:
                            # scan straight from PSUM
                            in0 = wq_ps[:].rearrange("p (w u) -> p w u", w=32)
                            if qn == "a":
                                scan_quarter(wq_ps, AV_A, 32, 0, in0)
                            elif qn == "b":
                                scan_quarter(wq_ps, AV_B, 32, 224, in0)
                            elif qn == "c":
                                scan_quarter(wq_ps, AV_C, 32, 256, in0)
                            else:
                                for k in range(3):
                                    scan_quarter(wq_ps, AV_D + 32 * k, 32,
                                                 32 + 32 * k, in0)

                    # ---- fp16 product + segmented-reduce paths ----
                    def seg_reduce(dst, src_ap):
                        with nc.allow_low_precision("fp16 page sums"):
                            nc.vector.tensor_reduce(
                                out=dst, in_=src_ap,
                                axis=mybir.AxisListType.X, op=OP.add,
                                opt_input=False, opt_output=False)

                    def prod_engine(name):
                        return nc.vector if name == "dve" else nc.gpsimd

                    if plan["ab"][0] == "pool":
                        cv = conv_tiles["ab"]
                        pr_ab = prd_p.tile([128, 2048], f16, tag="pab",
                                           name="pr_ab")
                        prod_engine(plan["ab"][1]).tensor_tensor(
                            out=pr_ab[:].rearrange("p (w h u) -> p w h u",
                                                   h=2, u=32),
                            in0=cv[:].rearrange("p (h w u) -> p w h u",
                                                h=2, u=32),
                            in1=av_sb[:, AV_A:AV_A + 64]
                                .rearrange("p (h u) -> p () h u", h=2)
                                .to_broadcast([128, 32, 2, 32]),
                            op=OP.mult)
                        seg_reduce(red[:, 0:32],
                                   pr_ab[:].rearrange("p (w v) -> p w v",
                                                      v=64))
                    if plan["c"][0] == "pool":
                        cv = conv_tiles["c"]
                        pr_c = prd_p.tile([128, 1024], f16, tag="pc",
                                          name="pr_c")
                        prod_engine(plan["c"][1]).tensor_tensor(
                            out=pr_c[:].rearrange("p (w u) -> p w u", w=32),
                            in0=cv[:].rearrange("p (w u) -> p w u", w=32),
                            in1=av_sb[:, AV_C:AV_C + 32]
                                .rearrange("p u -> p () u")
                                .to_broadcast([128, 32, 32]),
                            op=OP.mult)
                        seg_reduce(red[:, 256:288],
                                   pr_c[:].rearrange("p (w u) -> p w u",
                                                     w=32))
                    if plan["d"][0] == "pool":
                        cv = conv_tiles["d"]
                        pr_d = prd_p.tile([128, 3072], f16, tag="pd",
                                          name="pr_d")
                        prod_engine(plan["d"][1]).tensor_tensor(
                            out=pr_d[:].rearrange("p (k w u) -> p k w u",
                                                  k=3, u=32),
                            in0=cv[:].rearrange("p (w u) -> p () w u", w=32)
                                .to_broadcast([128, 3, 32, 32]),
                            in1=av_sb[:, AV_D:AV_D + 96]
                                .rearrange("p (k u) -> p k () u", k=3)
                                .to_broadcast([128, 3, 32, 32]),
                            op=OP.mult)
                        seg_reduce(red[:, 32:128],
                                   pr_d[:].rearrange("p (kw u) -> p kw u",
                                                     u=32))

                    # ---- c' = c-block * sh1_k (ACT, per-partition scale) --
                    for k in range(3):
                        nc.scalar.activation(
                            red[:, 128 + 32 * k:160 + 32 * k],
                            red[:, 256:288],
                            AF.Copy,
                            scale=sh1_sb[:, k:k + 1])

                    # ---- scatter (one lhsT; PSUM-accumulated over chunk;
                    #      exactly ONE start=True matmul per chunk) ----
                    nc.tensor.matmul(m_ps[:, 0:224], lhsT=s_sb[:],
                                     rhs=red[:, 0:224],
                                     start=first,
                                     stop=(last and plan["ab"][0] != "scan"),
                                     skip_group_check=True)
                    if plan["ab"][0] == "scan":
                        nc.tensor.matmul(m_ps[:, 0:32], lhsT=s_sb[:],
                                         rhs=red[:, 224:256],
                                         start=False, stop=last,
                                         skip_group_check=True)

                # ---- store chunk ----
                o_sb = out_p.tile([128, M_COLS], f32, tag="o")
                nc.scalar.copy(out=o_sb[:], in_=m_ps[:])
                nc.sync.dma_start(out_d[cc * 128:(cc + 1) * 128, :], o_sb[:])

    nc.compile()
    _PROGRAM_CACHE[key] = nc
    return nc


# --------------------------------------------------------------------------- #
# Entry point
# --------------------------------------------------------------------------- #
def _build_in_maps(prep):
    in_maps = []
    for c in range(NC):
        in_maps.append({
            "embT": prep["embT"][c],
            "avx": prep["avx"][c],
            "s16": prep["s16"][c],
            "sh1f": prep["sh1f"][c],
            "w1": prep["W1eff"],
            "w2": prep["W2eff"],
        })
    return in_maps


def _postprocess(per_core_out, assign, plan_s=None):
    plan = _parse_plan(plan_s if plan_s is not None else _DEFAULT_PLAN)
    assign, node_perm = assign
    M = np.empty((N_NODES, M_COLS), np.float32)
    for core in range(NC):
        for s in range(CHUNKS_PER_CORE):
            c = int(assign[core, s])
            M[c * NPC:(c + 1) * NPC] = per_core_out[core][s * NPC:(s + 1) * NPC]

    def blkdiff(B):
        # per-32-block de-cumsum along columns
        out = B.copy()
        out[:, 1:] -= B[:, :-1]
        return out

    out0 = M[:, 0:32].copy()
    dpart = M[:, 32:128].reshape(N_NODES, 3, 32)
    cpart = M[:, 128:224].reshape(N_NODES, 3, 32)
    if plan["ab"][0] == "scan":
        out0 = blkdiff(out0)          # cum(a)+cum(b) both reset at col 0
    if plan["d"][0] == "scan":
        dpart = np.concatenate(
            [blkdiff(dpart[:, k])[:, None, :] for k in range(3)], axis=1)
    if plan["c"][0] == "scan":
        cpart = np.concatenate(
            [blkdiff(cpart[:, k])[:, None, :] for k in range(3)], axis=1)
    out1 = dpart + cpart              # [N, 3, 32] (k, w)
    out_rel = np.empty((N_NODES, 128), np.float32)
    out_rel[:, :32] = out0
    out_rel[:, 32:] = out1.transpose(0, 2, 1).reshape(N_NODES, 96)  # (w, k)
    out = np.empty_like(out_rel)
    out[node_perm] = out_rel
    return out


def _prepare(x, edge_index, edge_attr, edge_len_emb, W1, W2, repeat=1,
             plan_s=None):
    if plan_s is None:
        plan_s = _DEFAULT_PLAN
    x = np.asarray(x, np.float32)
    edge_index = np.asarray(edge_index)
    edge_attr = np.asarray(edge_attr, np.float32)
    edge_len_emb = np.asarray(edge_len_emb, np.float32)
    W1 = np.asarray(W1, np.float32)
    W2 = np.asarray(W2, np.float32)
    prep = _host_prep(x, edge_index, edge_attr, edge_len_emb, W1, W2)
    nc = _build_program(prep["schedule"], prep["e_pad"], plan_s, repeat=repeat)
    return prep, nc, _build_in_maps(prep)


def kernel(x, edge_index, edge_attr, edge_len_emb, W1, W2, _results_out=None):
    prep, nc, in_maps = _prepare(x, edge_index, edge_attr, edge_len_emb,
                                 W1, W2)

    from concourse.bass_utils import run_bass_kernel_spmd

    res = run_bass_kernel_spmd(nc, in_maps, core_ids=list(range(NC)))
    if _results_out is not None:
        _results_out.append(res)

    return _postprocess([res.results[c]["out"] for c in range(NC)],
                        prep["assign"], _DEFAULT_PLAN)
